# revision 7
# baseline (speedup 1.0000x reference)
"""Trainium2 Bass kernel for nn_BertWordPair (sparse_attention).

Computes: y = x @ W1 + b1 -> split into (q_tok, q_utt, k_tok, k_utt) per
channel c in [0,3); RoPE with block-sign structure from seg_ids; output
logits [B, S, S, 3] = sum over the two groups of the selected-variant
bilinear forms.

Strategy (8 NeuronCores):
  - Data-parallel over batch (2) x query-row quarters (4): each core owns
    512 output rows of one batch and all 2048 columns.
  - On device, everything is computed in transposed layout (features on
    partitions): y^T tiles come straight out of the PE with the feature
    (RoPE) dim on partitions, ready to be the contraction dim of the
    logits matmul.  x^T, the W1 column permutation, and the RoPE
    cos/sin tables are precomputed on the host.
  - The per-(row-seg, col-seg) variant selection (pp / q_neg·k_pos /
    q_pos·k_neg) reduces to two signs: sigma_q (per column block, applied
    as a per-partition scalar when forming Q_eff) and sigma_k (per
    column, folded into the host-built SIN table for K).  Matmuls run in
    float32r (full-rate fp32 mode, moving dim 512 >= 256).
"""
import sys
sys.path.insert(0, '/opt/trn_rl_repo')

import numpy as np

B, S, H, C = 2, 2048, 768, 3
DG = 256            # rope dim per group (tok / utt)
D2 = 512            # concat feature dim per channel (tok 256 + utt 256)
N_CORES = 8
QUARTERS = 4
RPC = S // QUARTERS  # 512 rows per core
BLK = 512            # column block
NB = S // BLK        # 4
KH = H // 128        # 6 contraction tiles for dense1
FT = (C * D2) // 128  # 12 feature tiles (q or k side)
MT = RPC // 128      # 4 row tiles per core
DT = D2 // 128       # 4 d-tiles per channel


def _variant(s, t):
    # 0=PP, 1=NP (q_neg*k_pos), 2=PN (q_pos*k_neg)
    if s >= 1 and t > s:
        return 1
    if t >= 1 and s > t:
        return 2
    return 0


def _rope_tables(pos, base):
    """pos: [n] ints -> cos [256, n], sin_signed [256, n] (float32).
    Row d uses freq[d//2]; sin rows carry the rotate-half sign
    (-1 on even rows, +1 on odd rows)."""
    freq = np.power(float(base), -2.0 * np.arange(DG // 2, dtype=np.float64) / DG)
    ang = freq[:, None] * pos[None, :].astype(np.float64)   # [128, n]
    cos = np.repeat(np.cos(ang), 2, axis=0)
    sin = np.repeat(np.sin(ang), 2, axis=0)
    sgn = np.where((np.arange(2 * (DG // 2)) % 2) == 0, -1.0, 1.0)
    return cos.astype(np.float32), (sin * sgn[:, None]).astype(np.float32)


def _host_prep(x, W1, b1, token_index, utterance_index, seg_ids):
    """Build per-core input maps + check fast-path validity."""
    x = np.asarray(x, np.float32)
    W1 = np.asarray(W1, np.float32)
    b1 = np.asarray(b1, np.float32)
    token_index = np.asarray(token_index)
    utterance_index = np.asarray(utterance_index)
    seg_ids = np.asarray(seg_ids)

    # --- W1 / b1 permutation: f_new = c*512 + g*256 + d
    # original chunk order within each 1024: q_tok, q_utt, k_tok, k_utt
    qcols = np.concatenate([
        np.arange(c * 1024 + g * 256, c * 1024 + g * 256 + 256)
        for c in range(C) for g in range(2)])
    kcols = np.concatenate([
        np.arange(c * 1024 + 512 + g * 256, c * 1024 + 512 + g * 256 + 256)
        for c in range(C) for g in range(2)])
    WQ = np.ascontiguousarray(W1[:, qcols])
    WK = np.ascontiguousarray(W1[:, kcols])
    bQ = b1[qcols].astype(np.float32)
    bK = b1[kcols].astype(np.float32)
    swap = np.arange(C * D2) ^ 1
    bQs = bQ[swap]
    bKs = bK[swap]
    biasr = np.concatenate([bQ, bQs, bK, bKs]).reshape(48, 128)

    xT = np.ascontiguousarray(x.transpose(0, 2, 1))  # [B, 768, 2048]

    in_maps = []
    metas = []
    for core in range(N_CORES):
        b, qt = core // QUARTERS, core % QUARTERS
        rows = slice(qt * RPC, (qt + 1) * RPC)
        seg = seg_ids[b]
        s_vals = seg[rows]
        if not np.all(s_vals == s_vals[0]):
            raise NotImplementedError("fast path: core rows must share one seg")
        s = int(s_vals[0])

        # per-column variants for this core
        var = np.array([_variant(s, int(t)) for t in seg], np.int32)
        sigq_col = np.where(var == 1, -1.0, 1.0).astype(np.float32)
        sigk_col = np.where(var == 2, -1.0, 1.0).astype(np.float32)
        sigq_blk = np.empty(NB, np.float32)
        for nb in range(NB):
            blk = sigq_col[nb * BLK:(nb + 1) * BLK]
            if not np.all(blk == blk[0]):
                raise NotImplementedError("fast path: sigma_q must be block-uniform")
            sigq_blk[nb] = blk[0]

        # tables: rows 0:256 tok, 256:512 utt
        ct_q, st_q = _rope_tables(token_index[b, rows], 10000.0)
        cu_q, su_q = _rope_tables(utterance_index[b, rows], 15.0)
        COSQ = np.concatenate([ct_q, cu_q], axis=0)
        SINQ = np.concatenate([st_q, su_q], axis=0)
        ct_k, st_k = _rope_tables(token_index[b], 10000.0)
        cu_k, su_k = _rope_tables(utterance_index[b], 15.0)
        COSK = np.concatenate([ct_k, cu_k], axis=0)
        SINK = np.concatenate([st_k, su_k], axis=0) * sigk_col[None, :]
        SIGQ = np.repeat(sigq_blk[:, None], 128, axis=1)

        in_maps.append({
            "xT": xT[b],
            "WQ": WQ, "WK": WK, "BIASR": biasr,
            "COSQ": np.ascontiguousarray(COSQ), "SINQ": np.ascontiguousarray(SINQ),
            "COSK": np.ascontiguousarray(COSK), "SINK": np.ascontiguousarray(SINK),
            "SIGQ": SIGQ,
        })
        metas.append({"b": b, "qt": qt})
    return in_maps, metas


def _build_program(reps=0):
    """Build the (SPMD-uniform) Bass program.  reps>0 wraps the body in a
    hardware For_i loop for timing measurements."""
    import concourse.bass as bass
    import concourse.bacc as bacc
    import concourse.mybir as mybir
    import concourse.tile as tile
    from contextlib import ExitStack

    f32 = mybir.dt.float32
    f32r = mybir.dt.float32r
    AF = mybir.ActivationFunctionType
    OP = mybir.AluOpType

    nc = bacc.Bacc("TRN2", target_bir_lowering=False, debug=False,
                   num_devices=N_CORES)
    xT = nc.dram_tensor("xT", [H, S], f32r, kind="ExternalInput")
    WQ = nc.dram_tensor("WQ", [H, C * D2], f32r, kind="ExternalInput")
    WK = nc.dram_tensor("WK", [H, C * D2], f32r, kind="ExternalInput")
    BIASR = nc.dram_tensor("BIASR", [48, 128], f32, kind="ExternalInput")
    COSQ = nc.dram_tensor("COSQ", [D2, RPC], f32, kind="ExternalInput")
    SINQ = nc.dram_tensor("SINQ", [D2, RPC], f32, kind="ExternalInput")
    COSK = nc.dram_tensor("COSK", [D2, S], f32, kind="ExternalInput")
    SINK = nc.dram_tensor("SINK", [D2, S], f32, kind="ExternalInput")
    SIGQ = nc.dram_tensor("SIGQ", [NB, 128], f32, kind="ExternalInput")
    XQ = nc.dram_tensor("XQ", [H, RPC], f32r, kind="ExternalInput")
    OUT = nc.dram_tensor("OUT", [C, RPC, S], f32, kind="ExternalOutput")

    with tile.TileContext(nc) as tc, ExitStack() as ctx:
        wp = ctx.enter_context(tc.tile_pool(name="wp", bufs=6))
        xp = ctx.enter_context(tc.tile_pool(name="xp", bufs=6))
        tabp = ctx.enter_context(tc.tile_pool(name="tabp", bufs=5))
        biasp = ctx.enter_context(tc.tile_pool(name="biasp", bufs=53))
        aqp = ctx.enter_context(tc.tile_pool(name="aqp", bufs=12))
        qeffp = ctx.enter_context(tc.tile_pool(name="qeffp", bufs=12))
        keffp = ctx.enter_context(tc.tile_pool(name="keffp", bufs=12))
        tmpp = ctx.enter_context(tc.tile_pool(name="tmpp", bufs=2))
        outp = ctx.enter_context(tc.tile_pool(name="outp", bufs=3))
        pap = ctx.enter_context(tc.tile_pool(name="pap", bufs=3, space="PSUM"))
        pbp = ctx.enter_context(tc.tile_pool(name="pbp", bufs=4, space="PSUM"))

        # bias tiles [128, 1]: rows 0:12 bQ, 12:24 bQs, 24:36 bK, 36:48 bKs
        bias_t = []
        for i in range(48):
            bt = biasp.tile([128, 1], f32, name=f"bias{i}", tag="bias")
            nc.sync.dma_start(bt[:], BIASR[i, :][:, None])
            bias_t.append(bt)
        sigq_t = []
        for nbi in range(NB):
            st_ = biasp.tile([128, 1], f32, name=f"sigq{nbi}", tag="bias")
            nc.sync.dma_start(st_[:], SIGQ[nbi, :][:, None])
            sigq_t.append(st_)

        # ---------- phase Q: own 512 rows (host passes XQ = xT[:, own rows],
        # since per-core column offsets can't appear in a SPMD-uniform
        # program) ----------
        aq_t, bq_t = [], []
        # q tables: 4 d-tiles
        cosq_t, sinq_t = [], []
        for dti in range(DT):
            ct = tabp.tile([128, RPC], f32, name="ctq")
            nc.sync.dma_start(ct[:], COSQ[dti * 128:(dti + 1) * 128, :])
            st_ = tabp.tile([128, RPC], f32, name="stq")
            nc.sync.dma_start(st_[:], SINQ[dti * 128:(dti + 1) * 128, :])
            cosq_t.append(ct)
            sinq_t.append(st_)

        # WQ tiles
        wq_t = []
        for kh in range(KH):
            wt = wp.tile([128, C * D2], f32r, name="wt")
            nc.sync.dma_start(wt[:], WQ[kh * 128:(kh + 1) * 128, :])
            wq_t.append(wt)
        xq_t = []
        for kh in range(KH):
            xt_ = xp.tile([128, RPC], f32r, name="xt")
            nc.sync.dma_start(xt_[:], XQ[kh * 128:(kh + 1) * 128, :])
            xq_t.append(xt_)

        mm = nc.tensor.matmul
        for ft in range(FT):
            ps = pap.tile([128, RPC], f32, name="psa")
            for kh in range(KH):
                mm(ps[:], wq_t[kh][:, ft * 128:(ft + 1) * 128],
                   xq_t[kh][:], start=(kh == 0), stop=(kh == KH - 1))
            dti = ft % DT
            aq = aqp.tile([128, RPC], f32, name="aq")
            bq = aqp.tile([128, RPC], f32, name="bq")
            # a' on DVE, b' on DVE
            vb = tmpp.tile([128, RPC], f32, name="vb")
            nc.scalar.activation(vb[:], ps[:], AF.Copy)
            vs = tmpp.tile([128, RPC], f32, name="vs")
            nc.sync.dma_start(vs[0:128:2, :], vb[1:128:2, :])
            nc.sync.dma_start(vs[1:128:2, :], vb[0:128:2, :])
            nc.vector.scalar_tensor_tensor(
                aq[:], vb[:], bias_t[ft][:], cosq_t[dti][:],
                mybir.AluOpType.add, mybir.AluOpType.mult)
            nc.vector.scalar_tensor_tensor(
                bq[:], vs[:], bias_t[12 + ft][:], sinq_t[dti][:],
                mybir.AluOpType.add, mybir.AluOpType.mult)
            aq_t.append(aq)
            bq_t.append(bq)

        # WK tiles (reuse wp slots once WQ matmuls consumed them)
        wk_t = []
        for kh in range(KH):
            wt = wp.tile([128, C * D2], f32r, name="wt")
            nc.sync.dma_start(wt[:], WK[kh * 128:(kh + 1) * 128, :])
            wk_t.append(wt)

        # ---------- per column block ----------
        for nb in range(NB):
            cols = slice(nb * BLK, (nb + 1) * BLK)
            # Qeff tiles for this block
            qeff_t = []
            for ft in range(FT):
                qe = qeffp.tile([128, RPC], f32r, name="qe")
                nc.vector.scalar_tensor_tensor(
                    qe[:], bq_t[ft][:], sigq_t[nb][:], aq_t[ft][:],
                    mybir.AluOpType.mult, mybir.AluOpType.add)
                qeff_t.append(qe)

            # xT block + tables
            xb_t = []
            for kh in range(KH):
                xt_ = xp.tile([128, BLK], f32r, name="xt")
                nc.sync.dma_start(xt_[:], xT[kh * 128:(kh + 1) * 128, cols])
                xb_t.append(xt_)
            cosk_t, sink_t = [], []
            for dti in range(DT):
                ct = tabp.tile([128, BLK], f32, name="ctq")
                nc.sync.dma_start(ct[:], COSK[dti * 128:(dti + 1) * 128, cols])
                st_ = tabp.tile([128, BLK], f32, name="stq")
                nc.sync.dma_start(st_[:], SINK[dti * 128:(dti + 1) * 128, cols])
                cosk_t.append(ct)
                sink_t.append(st_)

            # stage A for K + epilogue -> Keff
            keff_t = []
            for ftk in range(FT):
                dti = ftk % DT
                ps = pap.tile([128, BLK], f32, name="psa")
                for kh in range(KH):
                    mm(ps[:], wk_t[kh][:, ftk * 128:(ftk + 1) * 128],
                       xb_t[kh][:],
                       start=(kh == 0), stop=(kh == KH - 1))
                vb = tmpp.tile([128, BLK], f32, name="vb")
                nc.scalar.activation(vb[:], ps[:], AF.Copy)
                vs = tmpp.tile([128, BLK], f32, name="vs")
                nc.sync.dma_start(vs[0:128:2, :], vb[1:128:2, :])
                nc.sync.dma_start(vs[1:128:2, :], vb[0:128:2, :])
                ak = tmpp.tile([128, BLK], f32, name="ak", bufs=2)
                bk = tmpp.tile([128, BLK], f32, name="bk", bufs=2)
                nc.vector.scalar_tensor_tensor(
                    ak[:], vb[:], bias_t[24 + ftk][:], cosk_t[dti][:],
                    mybir.AluOpType.add, mybir.AluOpType.mult)
                nc.vector.scalar_tensor_tensor(
                    bk[:], vs[:], bias_t[36 + ftk][:], sink_t[dti][:],
                    mybir.AluOpType.add, mybir.AluOpType.mult)
                ke = keffp.tile([128, BLK], f32r, name="ke")
                nc.gpsimd.tensor_add(ke[:], ak[:], bk[:])
                keff_t.append(ke)

            # stage B
            for c in range(C):
                for m in range(MT):
                    pb = pbp.tile([128, BLK], f32, name="psb")
                    for dti in range(DT):
                        ftk = c * DT + dti
                        mm(pb[:],
                           qeff_t[ftk][:, m * 128:(m + 1) * 128],
                           keff_t[ftk][:],
                           start=(dti == 0), stop=(dti == DT - 1))
                    ob = outp.tile([128, BLK], f32, name="ob")
                    nc.scalar.activation(ob[:], pb[:], AF.Copy)
                    nc.sync.dma_start(
                        OUT[c, m * 128:(m + 1) * 128, cols], ob[:])

    nc.compile()
    return nc


_PROG_CACHE = {}


def kernel(**inputs):
    from concourse.bass_utils import run_bass_kernel_spmd

    in_maps, metas = _host_prep(**inputs)
    # per-core XQ (own columns of xT)
    for core in range(N_CORES):
        qt = metas[core]["qt"]
        m = in_maps[core]
        m["XQ"] = np.ascontiguousarray(m["xT"][:, qt * RPC:(qt + 1) * RPC])

    if "prog" not in _PROG_CACHE:
        _PROG_CACHE["prog"] = _build_program()
    nc = _PROG_CACHE["prog"]

    res = run_bass_kernel_spmd(nc, in_maps, list(range(N_CORES)))
    out = np.empty((B, S, S, C), np.float32)
    for core in range(N_CORES):
        b, qt = metas[core]["b"], metas[core]["qt"]
        o = res.results[core]["OUT"]  # [C, RPC, S]
        out[b, qt * RPC:(qt + 1) * RPC] = o.transpose(1, 2, 0)
    return out


# revision 19
# speedup vs baseline: 1.2119x; 1.2119x over previous
"""Trainium2 Bass kernel for nn_BertWordPair (sparse_attention).

Computes: y = x @ W1 + b1 -> split into (q_tok, q_utt, k_tok, k_utt) per
channel c in [0,3); RoPE with block-sign structure from seg_ids; output
logits [B, S, S, 3] = sum over the two groups of the selected-variant
bilinear forms.

Strategy (8 NeuronCores):
  - Data-parallel over batch (2) x query-row quarters (4): each core owns
    512 output rows of one batch and all 2048 columns.
  - Everything on device runs in transposed layout (features on
    partitions): y^T tiles come straight out of the PE with the feature
    (RoPE) dim on partitions, ready to be the contraction dim of the
    logits matmul.  x^T, the W1 column permutation, and the RoPE
    cos/sin tables are precomputed on the host in partition-major packed
    layouts so each logical load is a single DMA.
  - The per-(row-seg, col-seg) variant selection (pp / q_neg.k_pos /
    q_pos.k_neg) reduces to two signs: sigma_q (per column block, applied
    as a per-partition scalar when forming Q_eff) and sigma_k (per
    column, folded into the host-built SIN table for K).  Matmuls run in
    float32r (full-rate fp32 mode, moving dim 512 >= 256).
"""
import sys
sys.path.insert(0, '/opt/trn_rl_repo')

import numpy as np

B, S, H, C = 2, 2048, 768, 3
DG = 256             # rope dim per group (tok / utt)
D2 = 512             # concat feature dim per channel (tok 256 + utt 256)
N_CORES = 8
QUARTERS = 4
RPC = S // QUARTERS  # 512 rows per core
BLK = 512            # column block
NB = S // BLK        # 4
KH = H // 128        # 6 contraction tiles for dense1
FT = (C * D2) // 128  # 12 feature tiles (q or k side)
MT = RPC // 128      # 4 row tiles per core
DT = D2 // 128       # 4 d-tiles per channel


def _variant(s, t):
    # 0=PP, 1=NP (q_neg*k_pos), 2=PN (q_pos*k_neg)
    if s >= 1 and t > s:
        return 1
    if t >= 1 and s > t:
        return 2
    return 0


def _rope_tables(pos, base):
    """pos: [n] ints -> cos [256, n], sin_signed [256, n] (float32).
    Row d uses freq[d//2]; sin rows carry the rotate-half sign
    (-1 on even rows, +1 on odd rows)."""
    freq = np.power(float(base), -2.0 * np.arange(DG // 2, dtype=np.float64) / DG)
    ang = freq[:, None] * pos[None, :].astype(np.float64)   # [128, n]
    cos = np.repeat(np.cos(ang), 2, axis=0)
    sin = np.repeat(np.sin(ang), 2, axis=0)
    sgn = np.where((np.arange(2 * (DG // 2)) % 2) == 0, -1.0, 1.0)
    return cos.astype(np.float32), (sin * sgn[:, None]).astype(np.float32)


def _pack_pmajor(a, nt):
    """[nt*128, F] -> [128, nt, F] (partition-major chunks)."""
    F = a.shape[1]
    return np.ascontiguousarray(a.reshape(nt, 128, F).transpose(1, 0, 2))


def _host_prep(x, W1, b1, token_index, utterance_index, seg_ids):
    """Build per-core input maps + check fast-path validity."""
    x = np.asarray(x, np.float32)
    W1 = np.asarray(W1, np.float32)
    b1 = np.asarray(b1, np.float32)
    token_index = np.asarray(token_index)
    utterance_index = np.asarray(utterance_index)
    seg_ids = np.asarray(seg_ids)

    # --- W1 / b1 permutation: f_new = c*512 + g*256 + d
    qcols = np.concatenate([
        np.arange(c * 1024 + g * 256, c * 1024 + g * 256 + 256)
        for c in range(C) for g in range(2)])
    kcols = np.concatenate([
        np.arange(c * 1024 + 512 + g * 256, c * 1024 + 512 + g * 256 + 256)
        for c in range(C) for g in range(2)])
    WQp = _pack_pmajor(np.ascontiguousarray(W1[:, qcols]), KH)  # [128, KH, 1536]
    WKp = _pack_pmajor(np.ascontiguousarray(W1[:, kcols]), KH)
    bQ = b1[qcols].astype(np.float32)
    bK = b1[kcols].astype(np.float32)
    swap = np.arange(C * D2) ^ 1
    biasc = np.stack([bQ, bQ[swap], bK, bK[swap]]).reshape(4 * FT, 128).T
    biasc = np.ascontiguousarray(biasc)  # [128, 48]

    xT = x.transpose(0, 2, 1)  # [B, 768, 2048] (view)
    xTp = [_pack_pmajor(np.ascontiguousarray(xT[b]), KH) for b in range(B)]

    in_maps = []
    metas = []
    for core in range(N_CORES):
        b, qt = core // QUARTERS, core % QUARTERS
        rows = slice(qt * RPC, (qt + 1) * RPC)
        seg = seg_ids[b]
        s_vals = seg[rows]
        if not np.all(s_vals == s_vals[0]):
            raise NotImplementedError("fast path: core rows must share one seg")
        s = int(s_vals[0])

        var = np.array([_variant(s, int(t)) for t in seg], np.int32)
        sigq_col = np.where(var == 1, -1.0, 1.0).astype(np.float32)
        sigk_col = np.where(var == 2, -1.0, 1.0).astype(np.float32)
        sigq_blk = np.empty(NB, np.float32)
        for nb in range(NB):
            blk = sigq_col[nb * BLK:(nb + 1) * BLK]
            if not np.all(blk == blk[0]):
                raise NotImplementedError("fast path: sigma_q must be block-uniform")
            sigq_blk[nb] = blk[0]

        ct_q, st_q = _rope_tables(token_index[b, rows], 10000.0)
        cu_q, su_q = _rope_tables(utterance_index[b, rows], 15.0)
        ct_k, st_k = _rope_tables(token_index[b], 10000.0)
        cu_k, su_k = _rope_tables(utterance_index[b], 15.0)
        COSQ = np.concatenate([ct_q, cu_q], axis=0)       # [512, RPC]
        SINQ = np.concatenate([st_q, su_q], axis=0)
        COSK = np.concatenate([ct_k, cu_k], axis=0)       # [512, S]
        SINK = np.concatenate([st_k, su_k], axis=0) * sigk_col[None, :]
        # packed tables: [128, 2(cos/sin), DT, n]
        TABQ = np.stack([_pack_pmajor(COSQ, DT), _pack_pmajor(SINQ, DT)], axis=1)
        TABK = np.stack([_pack_pmajor(COSK, DT), _pack_pmajor(SINK, DT)], axis=1)
        SIGC = np.repeat(sigq_blk[None, :], 128, axis=0)  # [128, NB]

        in_maps.append({
            "XT": xTp[b],
            "XQ": np.ascontiguousarray(xTp[b][:, :, rows]),
            "WQ": WQp, "WK": WKp, "BIASC": biasc,
            "TABQ": np.ascontiguousarray(TABQ),
            "TABK": np.ascontiguousarray(TABK),
            "SIGC": np.ascontiguousarray(SIGC),
        })
        metas.append({"b": b, "qt": qt})
    return in_maps, metas


def _build_program(reps=0):
    """Build the SPMD-uniform Bass program."""
    import concourse.bacc as bacc
    import concourse.mybir as mybir
    import concourse.tile as tile
    from contextlib import ExitStack

    f32 = mybir.dt.float32
    f32r = mybir.dt.float32r
    AF = mybir.ActivationFunctionType
    OP = mybir.AluOpType

    nc = bacc.Bacc("TRN2", target_bir_lowering=False, debug=False,
                   num_devices=N_CORES)
    XT = nc.dram_tensor("XT", [128, KH, S], f32r, kind="ExternalInput")
    XQd = nc.dram_tensor("XQ", [128, KH, RPC], f32r, kind="ExternalInput")
    WQd = nc.dram_tensor("WQ", [128, KH, C * D2], f32r, kind="ExternalInput")
    WKd = nc.dram_tensor("WK", [128, KH, C * D2], f32r, kind="ExternalInput")
    BIASC = nc.dram_tensor("BIASC", [128, 4 * FT], f32, kind="ExternalInput")
    TABQ = nc.dram_tensor("TABQ", [128, 2, DT, RPC], f32, kind="ExternalInput")
    TABK = nc.dram_tensor("TABK", [128, 2, DT, S], f32, kind="ExternalInput")
    SIGC = nc.dram_tensor("SIGC", [128, NB], f32, kind="ExternalInput")
    OUT = nc.dram_tensor("OUT", [C, RPC, S], f32, kind="ExternalOutput")

    HB = 2                    # f-tiles per swap group (half a channel)
    NG = FT // HB             # swap groups per side (6)

    with tile.TileContext(nc) as tc, ExitStack() as ctx:
        wp = ctx.enter_context(tc.tile_pool(name="wp", bufs=3))
        xp = ctx.enter_context(tc.tile_pool(name="xp", bufs=2))
        tabp = ctx.enter_context(tc.tile_pool(name="tabp", bufs=2))
        biasp = ctx.enter_context(tc.tile_pool(name="biasp", bufs=1))
        aqp = ctx.enter_context(tc.tile_pool(name="aqp", bufs=12))
        qeffp = ctx.enter_context(tc.tile_pool(name="qeffp", bufs=6))
        keffp = ctx.enter_context(tc.tile_pool(name="keffp", bufs=6))
        vbp = ctx.enter_context(tc.tile_pool(name="vbp", bufs=2))
        outp = ctx.enter_context(tc.tile_pool(name="outp", bufs=3))
        pap = ctx.enter_context(tc.tile_pool(name="pap", bufs=3, space="PSUM"))
        pbp = ctx.enter_context(tc.tile_pool(name="pbp", bufs=4, space="PSUM"))

        bias_all = biasp.tile([128, 4 * FT], f32, name="bias_all")
        nc.sync.dma_start(bias_all[:], BIASC[:])
        sig_all = biasp.tile([128, NB], f32, name="sig_all")
        nc.sync.dma_start(sig_all[:], SIGC[:])

        mm = nc.tensor.matmul

        def stage_a(w_half, xtile, ft, psum, n):
            # w_half: three tiles [128, KH, 512]; xtile [128, KH, n]
            half, fo = divmod(ft, FT // 3)
            for kh in range(KH):
                mm(psum[:],
                   w_half[half][:, kh, fo * 128:(fo + 1) * 128],
                   xtile[:, kh, :],
                   start=(kh == 0), stop=(kh == KH - 1))

        def epilogue(psums, tab, bias_off, n, pool, group, ab_bufs=None):
            """psums: HB psum tiles for f-tiles [group*HB, group*HB+HB).
            Returns [(a, b)] tiles: a = (y+bias)*cos, b = (swap(y)+bias_s)*sin."""
            vb = vbp.tile([128, HB, n], f32, name="vb", tag="vb")
            for i in range(HB):
                nc.scalar.activation(vb[:, i, :], psums[i][:], AF.Copy)
            vs = vbp.tile([128, HB, n], f32, name="vs", tag="vs")
            nc.scalar.dma_start(vs[0:128:2, :, :], vb[1:128:2, :, :])
            nc.scalar.dma_start(vs[1:128:2, :, :], vb[0:128:2, :, :])
            res = []
            for i in range(HB):
                ft = group * HB + i
                dti = ft % DT
                a = pool.tile([128, n], f32, name="ea", tag="ea", bufs=ab_bufs)
                bb = pool.tile([128, n], f32, name="eb", tag="eb", bufs=ab_bufs)
                nc.vector.scalar_tensor_tensor(
                    a[:], vb[:, i, :],
                    bias_all[:, bias_off + ft:bias_off + ft + 1],
                    tab[:, 0, dti, :], OP.add, OP.mult)
                nc.vector.scalar_tensor_tensor(
                    bb[:], vs[:, i, :],
                    bias_all[:, bias_off + FT + ft:bias_off + FT + ft + 1],
                    tab[:, 1, dti, :], OP.add, OP.mult)
                res.append((a, bb))
            return res

        # ---------- phase Q ----------
        xq = xp.tile([128, KH, RPC], f32r, name="xq", tag="xb")
        nc.sync.dma_start(xq[:], XQd[:])
        tabq = tabp.tile([128, 2, DT, RPC], f32, name="tabq", tag="tab")
        nc.sync.dma_start(tabq[:], TABQ[:])
        wq_t = []
        for half in range(3):
            wt = wp.tile([128, KH, (C * D2) // 3], f32r, name="wt", tag="wt")
            nc.sync.dma_start(wt[:], WQd[:, :, half * 512:(half + 1) * 512])
            wq_t.append(wt)

        ab_q = []  # (a, b) per f-tile
        for g in range(NG):
            psums = []
            for i in range(HB):
                ps = pap.tile([128, RPC], f32, name="psa")
                stage_a(wq_t, xq, g * HB + i, ps, RPC)
                psums.append(ps)
            ab_q += epilogue(psums, tabq, 0, RPC, aqp, g)

        # WK tiles (reuse wp slots)
        wk_t = []
        for half in range(3):
            wt = wp.tile([128, KH, (C * D2) // 3], f32r, name="wt", tag="wt")
            nc.sync.dma_start(wt[:], WKd[:, :, half * 512:(half + 1) * 512])
            wk_t.append(wt)

        # ---------- per column block ----------
        for nb in range(NB):
            cols = slice(nb * BLK, (nb + 1) * BLK)
            sig = sig_all[:, nb:nb + 1]
            xb = xp.tile([128, KH, BLK], f32r, name="xb", tag="xb")
            nc.sync.dma_start(xb[:], XT[:, :, cols])
            tabk = tabp.tile([128, 2, DT, BLK], f32, name="tabk", tag="tab")
            nc.sync.dma_start(tabk[:], TABK[:, :, :, cols])

            # per channel: stage-A (2 groups) -> epilogue -> Keff, then
            # Q_eff + stage-B for that channel.  Interleaving A and B per
            # channel keeps every pool's live-set small (no slot deadlock).
            for c in range(C):
                keff_c = []
                for g in (2 * c, 2 * c + 1):
                    psums = []
                    for i in range(HB):
                        ps = pap.tile([128, BLK], f32, name="psa")
                        stage_a(wk_t, xb, g * HB + i, ps, BLK)
                        psums.append(ps)
                    ab = epilogue(psums, tabk, 2 * FT, BLK, keffp, g, ab_bufs=2)
                    for (a, bb) in ab:
                        ke = keffp.tile([128, BLK], f32r, name="ke", tag="ke",
                                        bufs=5)
                        nc.gpsimd.tensor_add(ke[:], a[:], bb[:])
                        keff_c.append(ke)
                qeff_c = []
                for dti in range(DT):
                    a, bb = ab_q[c * DT + dti]
                    qe = qeffp.tile([128, RPC], f32r, name="qe", tag="qe")
                    nc.vector.scalar_tensor_tensor(
                        qe[:], bb[:], sig, a[:], OP.mult, OP.add)
                    qeff_c.append(qe)
                for m in range(MT):
                    pb = pbp.tile([128, BLK], f32, name="psb")
                    for dti in range(DT):
                        mm(pb[:],
                           qeff_c[dti][:, m * 128:(m + 1) * 128],
                           keff_c[dti][:],
                           start=(dti == 0), stop=(dti == DT - 1))
                    ob = outp.tile([128, BLK], f32, name="ob", tag="ob")
                    nc.scalar.activation(ob[:], pb[:], AF.Copy)
                    nc.sync.dma_start(
                        OUT[c, m * 128:(m + 1) * 128, cols], ob[:])

    nc.compile()
    return nc


_PROG_CACHE = {}


def kernel(**inputs):
    from concourse.bass_utils import run_bass_kernel_spmd

    in_maps, metas = _host_prep(**inputs)
    if "prog" not in _PROG_CACHE:
        _PROG_CACHE["prog"] = _build_program()
    nc = _PROG_CACHE["prog"]

    res = run_bass_kernel_spmd(nc, in_maps, list(range(N_CORES)))
    out = np.empty((B, S, S, C), np.float32)
    for core in range(N_CORES):
        b, qt = metas[core]["b"], metas[core]["qt"]
        o = res.results[core]["OUT"]  # [C, RPC, S]
        out[b, qt * RPC:(qt + 1) * RPC] = o.transpose(1, 2, 0)
    return out


# revision 22
# speedup vs baseline: 1.3879x; 1.1453x over previous
"""Trainium2 Bass kernel for nn_BertWordPair (sparse_attention).

Computes: y = x @ W1 + b1 -> split into (q_tok, q_utt, k_tok, k_utt) per
channel c in [0,3); RoPE with block-sign structure from seg_ids; output
logits [B, S, S, 3] = sum over the two groups of the selected-variant
bilinear forms.

Strategy (8 NeuronCores):
  - Data-parallel over batch (2) x query-row quarters (4): each core owns
    512 output rows of one batch and all 2048 columns.
  - Everything on device runs in transposed layout (features on
    partitions): y^T tiles come straight out of the PE with the feature
    (RoPE) dim on partitions, ready to be the contraction dim of the
    logits matmul.  x^T, the W1 column permutation, and the RoPE tables
    are precomputed on the host in partition-major packed layouts so
    each logical load is one DMA.
  - RoPE rotate-half never crosses partitions: the host permutes W1
    columns so even-d and odd-d features live in separate 128-partition
    tiles (pair (2k, 2k+1) sits at partition k of the even/odd tile).
    rope(v)_e = v_e cos - v_o sin, rope(v)_o = v_o cos + v_e sin become
    four fused (bias+table) DVE ops + an add/sub on GPSIMD.
  - The per-(row-seg, col-seg) variant selection (pp / q_neg.k_pos /
    q_pos.k_neg) reduces to two signs: sigma_q (per column block, a
    per-partition scalar when forming Q_eff) and sigma_k (per column,
    folded into the host-built SIN table for K).  Matmuls run in
    float32r (full-rate fp32, moving dim 512 >= 256).
"""
import sys
sys.path.insert(0, '/opt/trn_rl_repo')

import numpy as np

B, S, H, C = 2, 2048, 768, 3
DG = 256             # rope dim per group (tok / utt)
D2 = 512             # feature dim per channel (tok 256 + utt 256)
N_CORES = 8
QUARTERS = 4
RPC = S // QUARTERS  # 512 rows per core
BLK = 512            # column block
NB = S // BLK        # 4
KH = H // 128        # 6 contraction tiles for dense1
FT = (C * D2) // 128  # 12 feature tiles per side (q or k)
MT = RPC // 128      # 4 row tiles per core
DT = D2 // 128       # 4 d-tiles per channel: tok-e, tok-o, utt-e, utt-o


def _variant(s, t):
    # 0=PP, 1=NP (q_neg*k_pos), 2=PN (q_pos*k_neg)
    if s >= 1 and t > s:
        return 1
    if t >= 1 and s > t:
        return 2
    return 0


def _rope_tables_half(pos, base):
    """pos: [n] ints -> cos [128, n], sin [128, n]; row k = freq k."""
    freq = np.power(float(base), -2.0 * np.arange(DG // 2, dtype=np.float64) / DG)
    ang = freq[:, None] * pos[None, :].astype(np.float64)
    return np.cos(ang).astype(np.float32), np.sin(ang).astype(np.float32)


def _perm_cols(side_off):
    """New feature order: c*512 + g*256 + p*128 + k  <-  orig
    c*1024 + side_off + g*256 + 2k + p."""
    cols = np.empty(C * D2, np.int64)
    f = 0
    for c in range(C):
        for g in range(2):
            for p in range(2):
                base = c * 1024 + side_off + g * 256 + p
                cols[f:f + 128] = base + 2 * np.arange(128)
                f += 128
    return cols


def _pack_pmajor(a, nt):
    """[nt*128, F] -> [128, nt, F] (partition-major chunks)."""
    F = a.shape[1]
    return np.ascontiguousarray(a.reshape(nt, 128, F).transpose(1, 0, 2))


def _host_prep(x, W1, b1, token_index, utterance_index, seg_ids):
    """Build per-core input maps + check fast-path validity."""
    x = np.asarray(x, np.float32)
    W1 = np.asarray(W1, np.float32)
    b1 = np.asarray(b1, np.float32)
    token_index = np.asarray(token_index)
    utterance_index = np.asarray(utterance_index)
    seg_ids = np.asarray(seg_ids)

    qcols = _perm_cols(0)     # q_tok at +0, q_utt at +256
    kcols = _perm_cols(512)   # k_tok at +512, k_utt at +768
    WQp = _pack_pmajor(np.ascontiguousarray(W1[:, qcols]), KH)  # [128, KH, 1536]
    WKp = _pack_pmajor(np.ascontiguousarray(W1[:, kcols]), KH)
    bQ = b1[qcols].astype(np.float32)
    bK = b1[kcols].astype(np.float32)
    biasc = np.ascontiguousarray(
        np.concatenate([bQ, bK]).reshape(2 * FT, 128).T)  # [128, 24]

    xT = x.transpose(0, 2, 1)
    xTp = [_pack_pmajor(np.ascontiguousarray(xT[b]), KH) for b in range(B)]

    in_maps = []
    metas = []
    for core in range(N_CORES):
        b, qt = core // QUARTERS, core % QUARTERS
        rows = slice(qt * RPC, (qt + 1) * RPC)
        seg = seg_ids[b]
        s_vals = seg[rows]
        if not np.all(s_vals == s_vals[0]):
            raise NotImplementedError("fast path: core rows must share one seg")
        s = int(s_vals[0])

        var = np.array([_variant(s, int(t)) for t in seg], np.int32)
        sigq_col = np.where(var == 1, -1.0, 1.0).astype(np.float32)
        sigk_col = np.where(var == 2, -1.0, 1.0).astype(np.float32)
        sigq_blk = np.empty(NB, np.float32)
        for nb in range(NB):
            blk = sigq_col[nb * BLK:(nb + 1) * BLK]
            if not np.all(blk == blk[0]):
                raise NotImplementedError("fast path: sigma_q must be block-uniform")
            sigq_blk[nb] = blk[0]

        ct_q, st_q = _rope_tables_half(token_index[b, rows], 10000.0)
        cu_q, su_q = _rope_tables_half(utterance_index[b, rows], 15.0)
        ct_k, st_k = _rope_tables_half(token_index[b], 10000.0)
        cu_k, su_k = _rope_tables_half(utterance_index[b], 15.0)
        # [128, 2(cos/sin), 2(tok/utt), n]
        TABQ = np.ascontiguousarray(np.stack(
            [np.stack([ct_q, cu_q], 0), np.stack([st_q, su_q], 0)], 0
        ).transpose(2, 0, 1, 3))
        TABK = np.ascontiguousarray(np.stack(
            [np.stack([ct_k, cu_k], 0),
             np.stack([st_k * sigk_col[None, :], su_k * sigk_col[None, :]], 0)], 0
        ).transpose(2, 0, 1, 3))
        # [128, 2, NB]: [:,0]=-sigma_q (even), [:,1]=+sigma_q (odd)
        SIGC = np.ascontiguousarray(np.broadcast_to(
            np.stack([-sigq_blk, sigq_blk], 0)[None, :, :], (128, 2, NB)).copy())

        in_maps.append({
            "XT": xTp[b],
            "XQ": np.ascontiguousarray(xTp[b][:, :, rows]),
            "WQ": WQp, "WK": WKp, "BIASC": biasc,
            "TABQ": TABQ, "TABK": TABK, "SIGC": SIGC,
        })
        metas.append({"b": b, "qt": qt})
    return in_maps, metas


def _build_program(reps=0):
    """Build the SPMD-uniform Bass program."""
    import concourse.bacc as bacc
    import concourse.mybir as mybir
    import concourse.tile as tile
    from contextlib import ExitStack

    f32 = mybir.dt.float32
    f32r = mybir.dt.float32r
    AF = mybir.ActivationFunctionType
    OP = mybir.AluOpType

    nc = bacc.Bacc("TRN2", target_bir_lowering=False, debug=False,
                   num_devices=N_CORES)
    XT = nc.dram_tensor("XT", [128, KH, S], f32r, kind="ExternalInput")
    XQd = nc.dram_tensor("XQ", [128, KH, RPC], f32r, kind="ExternalInput")
    WQd = nc.dram_tensor("WQ", [128, KH, C * D2], f32r, kind="ExternalInput")
    WKd = nc.dram_tensor("WK", [128, KH, C * D2], f32r, kind="ExternalInput")
    BIASC = nc.dram_tensor("BIASC", [128, 2 * FT], f32, kind="ExternalInput")
    TABQ = nc.dram_tensor("TABQ", [128, 2, 2, RPC], f32, kind="ExternalInput")
    TABK = nc.dram_tensor("TABK", [128, 2, 2, S], f32, kind="ExternalInput")
    SIGC = nc.dram_tensor("SIGC", [128, 2, NB], f32, kind="ExternalInput")
    OUT = nc.dram_tensor("OUT", [C, RPC, S], f32, kind="ExternalOutput")

    with tile.TileContext(nc) as tc, ExitStack() as ctx:
        wp = ctx.enter_context(tc.tile_pool(name="wp", bufs=3))
        xp = ctx.enter_context(tc.tile_pool(name="xp", bufs=2))
        tabp = ctx.enter_context(tc.tile_pool(name="tabp", bufs=2))
        biasp = ctx.enter_context(tc.tile_pool(name="biasp", bufs=1))
        aqp = ctx.enter_context(tc.tile_pool(name="aqp", bufs=6))
        qeffp = ctx.enter_context(tc.tile_pool(name="qeffp", bufs=6))
        keffp = ctx.enter_context(tc.tile_pool(name="keffp", bufs=8))
        outp = ctx.enter_context(tc.tile_pool(name="outp", bufs=3))
        pap = ctx.enter_context(tc.tile_pool(name="pap", bufs=4, space="PSUM"))
        pbp = ctx.enter_context(tc.tile_pool(name="pbp", bufs=4, space="PSUM"))

        bias_all = biasp.tile([128, 2 * FT], f32, name="bias_all")
        nc.sync.dma_start(bias_all[:], BIASC[:])
        sig_all = biasp.tile([128, 2, NB], f32, name="sig_all")
        nc.sync.dma_start(sig_all[:], SIGC[:])

        mm = nc.tensor.matmul

        def stage_a(w_parts, xtile, ft, psum):
            third, fo = divmod(ft, FT // 3)
            for kh in range(KH):
                mm(psum[:],
                   w_parts[third][:, kh, fo * 128:(fo + 1) * 128],
                   xtile[:, kh, :],
                   start=(kh == 0), stop=(kh == KH - 1))

        def rope_pair(ps_e, ps_o, fe, tab, g, n, pool, ab_bufs=None):
            """Four fused (bias+table) products for a parity pair.
            rope_pos_e = ae - as_ ; rope_pos_o = ao + bo
            rope_neg_e = ae + as_ ; rope_neg_o = ao - bo"""
            cos = tab[:, 0, g, :]
            sin = tab[:, 1, g, :]
            be = bias_all[:, fe:fe + 1]
            bod = bias_all[:, fe + 1:fe + 2]
            ae = pool.tile([128, n], f32, name="ae", tag="ae", bufs=ab_bufs)
            bo = pool.tile([128, n], f32, name="bo", tag="bo", bufs=ab_bufs)
            as_ = pool.tile([128, n], f32, name="as_", tag="as_", bufs=ab_bufs)
            ao = pool.tile([128, n], f32, name="ao", tag="ao", bufs=ab_bufs)
            nc.vector.scalar_tensor_tensor(ae[:], ps_e[:], be, cos, OP.add, OP.mult)
            nc.vector.scalar_tensor_tensor(bo[:], ps_e[:], be, sin, OP.add, OP.mult)
            nc.vector.scalar_tensor_tensor(as_[:], ps_o[:], bod, sin, OP.add, OP.mult)
            nc.vector.scalar_tensor_tensor(ao[:], ps_o[:], bod, cos, OP.add, OP.mult)
            return ae, as_, ao, bo

        def emit_body():
            # ---------- phase Q ----------
            xq = xp.tile([128, KH, RPC], f32r, name="xq", tag="xb")
            nc.sync.dma_start(xq[:], XQd[:])
            tabq = tabp.tile([128, 2, 2, RPC], f32, name="tabq", tag="tab")
            nc.sync.dma_start(tabq[:], TABQ[:])
            wq_t = []
            for third in range(3):
                wt = wp.tile([128, KH, (C * D2) // 3], f32r, name="wt", tag="wt")
                nc.sync.dma_start(wt[:], WQd[:, :, third * 512:(third + 1) * 512])
                wq_t.append(wt)

            ab_q = []  # per pair: (ae, as_, ao, bo)
            for pr in range(FT // 2):
                ps_e = pap.tile([128, RPC], f32, name="psa")
                stage_a(wq_t, xq, 2 * pr, ps_e)
                ps_o = pap.tile([128, RPC], f32, name="psa")
                stage_a(wq_t, xq, 2 * pr + 1, ps_o)
                ab_q.append(rope_pair(ps_e, ps_o, 2 * pr, tabq, pr % 2, RPC, aqp))

            wk_t = []
            for third in range(3):
                wt = wp.tile([128, KH, (C * D2) // 3], f32r, name="wt", tag="wt")
                nc.sync.dma_start(wt[:], WKd[:, :, third * 512:(third + 1) * 512])
                wk_t.append(wt)

            # ---------- per column block ----------
            for nb in range(NB):
                cols = slice(nb * BLK, (nb + 1) * BLK)
                sig_e = sig_all[:, 0, nb:nb + 1]
                sig_o = sig_all[:, 1, nb:nb + 1]
                xb = xp.tile([128, KH, BLK], f32r, name="xb", tag="xb")
                nc.sync.dma_start(xb[:], XT[:, :, cols])
                tabk = tabp.tile([128, 2, 2, BLK], f32, name="tabk", tag="tab")
                nc.sync.dma_start(tabk[:], TABK[:, :, :, cols])

                for c in range(C):
                    keff_c = []
                    for g in range(2):           # tok pair, utt pair
                        ft_e = c * DT + 2 * g
                        ps_e = pap.tile([128, BLK], f32, name="psa")
                        stage_a(wk_t, xb, ft_e, ps_e)
                        ps_o = pap.tile([128, BLK], f32, name="psa")
                        stage_a(wk_t, xb, ft_e + 1, ps_o)
                        ae, as_, ao, bo = rope_pair(
                            ps_e, ps_o, FT + ft_e, tabk, g, BLK, keffp, ab_bufs=3)
                        ke_e = keffp.tile([128, BLK], f32r, name="ke", tag="ke")
                        nc.gpsimd.tensor_sub(ke_e[:], ae[:], as_[:])
                        ke_o = keffp.tile([128, BLK], f32r, name="ke", tag="ke")
                        nc.gpsimd.tensor_add(ke_o[:], ao[:], bo[:])
                        keff_c += [ke_e, ke_o]
                    qeff_c = []
                    for g in range(2):
                        ae, as_, ao, bo = ab_q[c * 2 + g]
                        qe_e = qeffp.tile([128, RPC], f32r, name="qe", tag="qe")
                        nc.vector.scalar_tensor_tensor(
                            qe_e[:], as_[:], sig_e, ae[:], OP.mult, OP.add)
                        qe_o = qeffp.tile([128, RPC], f32r, name="qe", tag="qe")
                        nc.vector.scalar_tensor_tensor(
                            qe_o[:], bo[:], sig_o, ao[:], OP.mult, OP.add)
                        qeff_c += [qe_e, qe_o]
                    for m in range(MT):
                        pb = pbp.tile([128, BLK], f32, name="psb")
                        for dti in range(DT):
                            mm(pb[:],
                               qeff_c[dti][:, m * 128:(m + 1) * 128],
                               keff_c[dti][:],
                               start=(dti == 0), stop=(dti == DT - 1))
                        ob = outp.tile([128, BLK], f32, name="ob", tag="ob")
                        nc.scalar.activation(ob[:], pb[:], AF.Copy)
                        nc.sync.dma_start(
                            OUT[c, m * 128:(m + 1) * 128, cols], ob[:])

        if reps and reps > 1:
            with tc.For_i(0, reps, 1):
                emit_body()
        else:
            emit_body()

    nc.compile()
    return nc


_PROG_CACHE = {}


def kernel(**inputs):
    from concourse.bass_utils import run_bass_kernel_spmd

    in_maps, metas = _host_prep(**inputs)
    if "prog" not in _PROG_CACHE:
        _PROG_CACHE["prog"] = _build_program()
    nc = _PROG_CACHE["prog"]

    res = run_bass_kernel_spmd(nc, in_maps, list(range(N_CORES)))
    out = np.empty((B, S, S, C), np.float32)
    for core in range(N_CORES):
        b, qt = metas[core]["b"], metas[core]["qt"]
        o = res.results[core]["OUT"]  # [C, RPC, S]
        out[b, qt * RPC:(qt + 1) * RPC] = o.transpose(1, 2, 0)
    return out


# revision 23
# speedup vs baseline: 1.4754x; 1.0630x over previous
"""Trainium2 Bass kernel for nn_BertWordPair (sparse_attention).

Computes: y = x @ W1 + b1 -> split into (q_tok, q_utt, k_tok, k_utt) per
channel c in [0,3); RoPE with block-sign structure from seg_ids; output
logits [B, S, S, 3] = sum over the two groups of the selected-variant
bilinear forms.

Strategy (8 NeuronCores):
  - Data-parallel over batch (2) x query-row quarters (4): each core owns
    512 output rows of one batch and all 2048 columns.
  - Everything on device runs in transposed layout (features on
    partitions): y^T tiles come straight out of the PE with the feature
    (RoPE) dim on partitions, ready to be the contraction dim of the
    logits matmul.  x^T, the W1 column permutation, and the RoPE tables
    are precomputed on the host in partition-major packed layouts so
    each logical load is one DMA.
  - RoPE rotate-half never crosses partitions: the host permutes W1
    columns so even-d and odd-d features live in separate 128-partition
    tiles (pair (2k, 2k+1) sits at partition k of the even/odd tile).
    rope(v)_e = v_e cos - v_o sin, rope(v)_o = v_o cos + v_e sin become
    four fused (bias+table) DVE ops + an add/sub on GPSIMD.
  - The per-(row-seg, col-seg) variant selection (pp / q_neg.k_pos /
    q_pos.k_neg) reduces to two signs: sigma_q (per column block, a
    per-partition scalar when forming Q_eff) and sigma_k (per column,
    folded into the host-built SIN table for K).  Matmuls run in
    float32r (full-rate fp32, moving dim 512 >= 256).
"""
import sys
sys.path.insert(0, '/opt/trn_rl_repo')

import numpy as np

B, S, H, C = 2, 2048, 768, 3
DG = 256             # rope dim per group (tok / utt)
D2 = 512             # feature dim per channel (tok 256 + utt 256)
N_CORES = 8
QUARTERS = 4
RPC = S // QUARTERS  # 512 rows per core
BLK = 512            # column block
NB = S // BLK        # 4
KH = H // 128        # 6 contraction tiles for dense1
FT = (C * D2) // 128  # 12 feature tiles per side (q or k)
MT = RPC // 128      # 4 row tiles per core
DT = D2 // 128       # 4 d-tiles per channel: tok-e, tok-o, utt-e, utt-o


def _variant(s, t):
    # 0=PP, 1=NP (q_neg*k_pos), 2=PN (q_pos*k_neg)
    if s >= 1 and t > s:
        return 1
    if t >= 1 and s > t:
        return 2
    return 0


def _rope_tables_half(pos, base):
    """pos: [n] ints -> cos [128, n], sin [128, n]; row k = freq k."""
    freq = np.power(float(base), -2.0 * np.arange(DG // 2, dtype=np.float64) / DG)
    ang = freq[:, None] * pos[None, :].astype(np.float64)
    return np.cos(ang).astype(np.float32), np.sin(ang).astype(np.float32)


def _perm_cols(side_off):
    """New feature order: c*512 + g*256 + p*128 + k  <-  orig
    c*1024 + side_off + g*256 + 2k + p."""
    cols = np.empty(C * D2, np.int64)
    f = 0
    for c in range(C):
        for g in range(2):
            for p in range(2):
                base = c * 1024 + side_off + g * 256 + p
                cols[f:f + 128] = base + 2 * np.arange(128)
                f += 128
    return cols


def _pack_pmajor(a, nt):
    """[nt*128, F] -> [128, nt, F] (partition-major chunks)."""
    F = a.shape[1]
    return np.ascontiguousarray(a.reshape(nt, 128, F).transpose(1, 0, 2))


def _host_prep(x, W1, b1, token_index, utterance_index, seg_ids):
    """Build per-core input maps + check fast-path validity."""
    x = np.asarray(x, np.float32)
    W1 = np.asarray(W1, np.float32)
    b1 = np.asarray(b1, np.float32)
    token_index = np.asarray(token_index)
    utterance_index = np.asarray(utterance_index)
    seg_ids = np.asarray(seg_ids)

    qcols = _perm_cols(0)     # q_tok at +0, q_utt at +256
    kcols = _perm_cols(512)   # k_tok at +512, k_utt at +768
    WQp = _pack_pmajor(np.ascontiguousarray(W1[:, qcols]), KH)  # [128, KH, 1536]
    WKp = _pack_pmajor(np.ascontiguousarray(W1[:, kcols]), KH)
    bQ = b1[qcols].astype(np.float32)
    bK = b1[kcols].astype(np.float32)
    biasc = np.ascontiguousarray(
        np.concatenate([bQ, bK]).reshape(2 * FT, 128).T)  # [128, 24]

    xT = x.transpose(0, 2, 1)
    xTp = [_pack_pmajor(np.ascontiguousarray(xT[b]), KH) for b in range(B)]

    in_maps = []
    metas = []
    for core in range(N_CORES):
        b, qt = core // QUARTERS, core % QUARTERS
        rows = slice(qt * RPC, (qt + 1) * RPC)
        seg = seg_ids[b]
        s_vals = seg[rows]
        if not np.all(s_vals == s_vals[0]):
            raise NotImplementedError("fast path: core rows must share one seg")
        s = int(s_vals[0])

        var = np.array([_variant(s, int(t)) for t in seg], np.int32)
        sigq_col = np.where(var == 1, -1.0, 1.0).astype(np.float32)
        sigk_col = np.where(var == 2, -1.0, 1.0).astype(np.float32)
        sigq_blk = np.empty(NB, np.float32)
        for nb in range(NB):
            blk = sigq_col[nb * BLK:(nb + 1) * BLK]
            if not np.all(blk == blk[0]):
                raise NotImplementedError("fast path: sigma_q must be block-uniform")
            sigq_blk[nb] = blk[0]

        ct_q, st_q = _rope_tables_half(token_index[b, rows], 10000.0)
        cu_q, su_q = _rope_tables_half(utterance_index[b, rows], 15.0)
        ct_k, st_k = _rope_tables_half(token_index[b], 10000.0)
        cu_k, su_k = _rope_tables_half(utterance_index[b], 15.0)
        # [128, 2(cos/sin), 2(tok/utt), n]
        TABQ = np.ascontiguousarray(np.stack(
            [np.stack([ct_q, cu_q], 0), np.stack([st_q, su_q], 0)], 0
        ).transpose(2, 0, 1, 3))
        TABK = np.ascontiguousarray(np.stack(
            [np.stack([ct_k, cu_k], 0),
             np.stack([st_k * sigk_col[None, :], su_k * sigk_col[None, :]], 0)], 0
        ).transpose(2, 0, 1, 3))
        # [128, 2, NB]: [:,0]=-sigma_q (even), [:,1]=+sigma_q (odd)
        SIGC = np.ascontiguousarray(np.broadcast_to(
            np.stack([-sigq_blk, sigq_blk], 0)[None, :, :], (128, 2, NB)).copy())

        in_maps.append({
            "XT": xTp[b],
            "XQ": np.ascontiguousarray(xTp[b][:, :, rows]),
            "WQ": WQp, "WK": WKp, "BIASC": biasc,
            "TABQ": TABQ, "TABK": TABK, "SIGC": SIGC,
        })
        metas.append({"b": b, "qt": qt})
    return in_maps, metas


def _build_program(reps=0):
    """Build the SPMD-uniform Bass program."""
    import concourse.bacc as bacc
    import concourse.mybir as mybir
    import concourse.tile as tile
    from contextlib import ExitStack

    f32 = mybir.dt.float32
    f32r = mybir.dt.float32r
    AF = mybir.ActivationFunctionType
    OP = mybir.AluOpType

    nc = bacc.Bacc("TRN2", target_bir_lowering=False, debug=False,
                   num_devices=N_CORES)
    XT = nc.dram_tensor("XT", [128, KH, S], f32r, kind="ExternalInput")
    XQd = nc.dram_tensor("XQ", [128, KH, RPC], f32r, kind="ExternalInput")
    WQd = nc.dram_tensor("WQ", [128, KH, C * D2], f32r, kind="ExternalInput")
    WKd = nc.dram_tensor("WK", [128, KH, C * D2], f32r, kind="ExternalInput")
    BIASC = nc.dram_tensor("BIASC", [128, 2 * FT], f32, kind="ExternalInput")
    TABQ = nc.dram_tensor("TABQ", [128, 2, 2, RPC], f32, kind="ExternalInput")
    TABK = nc.dram_tensor("TABK", [128, 2, 2, S], f32, kind="ExternalInput")
    SIGC = nc.dram_tensor("SIGC", [128, 2, NB], f32, kind="ExternalInput")
    OUT = nc.dram_tensor("OUT", [C, RPC, S], f32, kind="ExternalOutput")

    with tile.TileContext(nc) as tc, ExitStack() as ctx:
        wp = ctx.enter_context(tc.tile_pool(name="wp", bufs=4))
        xp = ctx.enter_context(tc.tile_pool(name="xp", bufs=2))
        tabp = ctx.enter_context(tc.tile_pool(name="tabp", bufs=2))
        biasp = ctx.enter_context(tc.tile_pool(name="biasp", bufs=1))
        aqp = ctx.enter_context(tc.tile_pool(name="aqp", bufs=6))
        qeffp = ctx.enter_context(tc.tile_pool(name="qeffp", bufs=6))
        keffp = ctx.enter_context(tc.tile_pool(name="keffp", bufs=8))
        outp = ctx.enter_context(tc.tile_pool(name="outp", bufs=3))
        pap = ctx.enter_context(tc.tile_pool(name="pap", bufs=4, space="PSUM"))
        pbp = ctx.enter_context(tc.tile_pool(name="pbp", bufs=4, space="PSUM"))

        bias_all = biasp.tile([128, 2 * FT], f32, name="bias_all")
        nc.sync.dma_start(bias_all[:], BIASC[:])
        sig_all = biasp.tile([128, 2, NB], f32, name="sig_all")
        nc.sync.dma_start(sig_all[:], SIGC[:])

        mm = nc.tensor.matmul

        def stage_a(w_parts, xtile, ft, psum):
            third, fo = divmod(ft, FT // 3)
            for kh in range(KH):
                mm(psum[:],
                   w_parts[third][:, kh, fo * 128:(fo + 1) * 128],
                   xtile[:, kh, :],
                   start=(kh == 0), stop=(kh == KH - 1))

        def rope_pair(ps_e, ps_o, fe, tab, g, n, pool, ab_bufs=None):
            """Four fused (bias+table) products for a parity pair.
            rope_pos_e = ae - as_ ; rope_pos_o = ao + bo
            rope_neg_e = ae + as_ ; rope_neg_o = ao - bo"""
            cos = tab[:, 0, g, :]
            sin = tab[:, 1, g, :]
            be = bias_all[:, fe:fe + 1]
            bod = bias_all[:, fe + 1:fe + 2]
            ae = pool.tile([128, n], f32, name="ae", tag="ae", bufs=ab_bufs)
            bo = pool.tile([128, n], f32, name="bo", tag="bo", bufs=ab_bufs)
            as_ = pool.tile([128, n], f32, name="as_", tag="as_", bufs=ab_bufs)
            ao = pool.tile([128, n], f32, name="ao", tag="ao", bufs=ab_bufs)
            nc.vector.scalar_tensor_tensor(ae[:], ps_e[:], be, cos, OP.add, OP.mult)
            nc.vector.scalar_tensor_tensor(bo[:], ps_e[:], be, sin, OP.add, OP.mult)
            nc.vector.scalar_tensor_tensor(as_[:], ps_o[:], bod, sin, OP.add, OP.mult)
            nc.vector.scalar_tensor_tensor(ao[:], ps_o[:], bod, cos, OP.add, OP.mult)
            return ae, as_, ao, bo

        def emit_body():
            # ---------- phase Q ----------
            xq = xp.tile([128, KH, RPC], f32r, name="xq", tag="xb")
            wq_t = [wp.tile([128, KH, (C * D2) // 3], f32r, name="wt", tag="wt")
                    for _ in range(3)]
            for kh in range(KH):
                nc.sync.dma_start(xq[:, kh, :], XQd[:, kh, :])
                for third in range(3):
                    nc.sync.dma_start(
                        wq_t[third][:, kh, :],
                        WQd[:, kh, third * 512:(third + 1) * 512])
            tabq = tabp.tile([128, 2, 2, RPC], f32, name="tabq", tag="tab")
            nc.sync.dma_start(tabq[:], TABQ[:])

            ab_q = []  # per pair: (ae, as_, ao, bo)
            for pr in range(FT // 2):
                ps_e = pap.tile([128, RPC], f32, name="psa")
                stage_a(wq_t, xq, 2 * pr, ps_e)
                ps_o = pap.tile([128, RPC], f32, name="psa")
                stage_a(wq_t, xq, 2 * pr + 1, ps_o)
                ab_q.append(rope_pair(ps_e, ps_o, 2 * pr, tabq, pr % 2, RPC, aqp))

            wk_t = []
            for third in range(3):
                wt = wp.tile([128, KH, (C * D2) // 3], f32r, name="wt", tag="wt")
                nc.sync.dma_start(wt[:], WKd[:, :, third * 512:(third + 1) * 512])
                wk_t.append(wt)

            # ---------- per column block ----------
            for nb in range(NB):
                cols = slice(nb * BLK, (nb + 1) * BLK)
                sig_e = sig_all[:, 0, nb:nb + 1]
                sig_o = sig_all[:, 1, nb:nb + 1]
                xb = xp.tile([128, KH, BLK], f32r, name="xb", tag="xb")
                nc.sync.dma_start(xb[:], XT[:, :, cols])
                tabk = tabp.tile([128, 2, 2, BLK], f32, name="tabk", tag="tab")
                nc.sync.dma_start(tabk[:], TABK[:, :, :, cols])

                def emit_a(c):
                    keff_c = []
                    for g in range(2):           # tok pair, utt pair
                        ft_e = c * DT + 2 * g
                        ps_e = pap.tile([128, BLK], f32, name="psa")
                        stage_a(wk_t, xb, ft_e, ps_e)
                        ps_o = pap.tile([128, BLK], f32, name="psa")
                        stage_a(wk_t, xb, ft_e + 1, ps_o)
                        ae, as_, ao, bo = rope_pair(
                            ps_e, ps_o, FT + ft_e, tabk, g, BLK, keffp, ab_bufs=3)
                        ke_e = keffp.tile([128, BLK], f32r, name="ke", tag="ke")
                        nc.gpsimd.tensor_sub(ke_e[:], ae[:], as_[:])
                        ke_o = keffp.tile([128, BLK], f32r, name="ke", tag="ke")
                        nc.gpsimd.tensor_add(ke_o[:], ao[:], bo[:])
                        keff_c += [ke_e, ke_o]
                    return keff_c

                def emit_b(c, keff_c):
                    qeff_c = []
                    for g in range(2):
                        ae, as_, ao, bo = ab_q[c * 2 + g]
                        qe_e = qeffp.tile([128, RPC], f32r, name="qe", tag="qe")
                        nc.vector.scalar_tensor_tensor(
                            qe_e[:], as_[:], sig_e, ae[:], OP.mult, OP.add)
                        qe_o = qeffp.tile([128, RPC], f32r, name="qe", tag="qe")
                        nc.vector.scalar_tensor_tensor(
                            qe_o[:], bo[:], sig_o, ao[:], OP.mult, OP.add)
                        qeff_c += [qe_e, qe_o]
                    for m in range(MT):
                        pb = pbp.tile([128, BLK], f32, name="psb")
                        for dti in range(DT):
                            mm(pb[:],
                               qeff_c[dti][:, m * 128:(m + 1) * 128],
                               keff_c[dti][:],
                               start=(dti == 0), stop=(dti == DT - 1))
                        ob = outp.tile([128, BLK], f32, name="ob", tag="ob")
                        nc.scalar.activation(ob[:], pb[:], AF.Copy)
                        nc.sync.dma_start(
                            OUT[c, m * 128:(m + 1) * 128, cols], ob[:])

                keffs = {0: emit_a(0)}
                for c in range(C):
                    if c + 1 < C:
                        keffs[c + 1] = emit_a(c + 1)
                    emit_b(c, keffs.pop(c))

        if reps and reps > 1:
            with tc.For_i(0, reps, 1):
                emit_body()
        else:
            emit_body()

    nc.compile()
    return nc


_PROG_CACHE = {}


def kernel(**inputs):
    from concourse.bass_utils import run_bass_kernel_spmd

    in_maps, metas = _host_prep(**inputs)
    if "prog" not in _PROG_CACHE:
        _PROG_CACHE["prog"] = _build_program()
    nc = _PROG_CACHE["prog"]

    res = run_bass_kernel_spmd(nc, in_maps, list(range(N_CORES)))
    out = np.empty((B, S, S, C), np.float32)
    for core in range(N_CORES):
        b, qt = metas[core]["b"], metas[core]["qt"]
        o = res.results[core]["OUT"]  # [C, RPC, S]
        out[b, qt * RPC:(qt + 1) * RPC] = o.transpose(1, 2, 0)
    return out
